# revision 14
# baseline (speedup 1.0000x reference)
"""Trainium2 Bass GRU kernel v13: all-DVE recurrent chain, legal PSUM reads.

Per core (32 sequences), transposed layout (hidden on partitions, batch on
free dim). State kept as a = z*h_prev and b = (1-z)*n so PSUM accumulation
performs h = a + b inside the recurrent matmuls. Design:

- r, z, n-recurrent preactivations live in SEPARATE PSUM banks: the tile
  framework sem-orders reads of the same PSUM tile (even on one engine,
  with the writer ack), so sharing a bank between SIGP and ZP7 costs
  160ns/step. 6 recurrent matmuls/step; PE is far from busy.
- HW constraint: a DVE op may read at most ONE input from PSUM. The chain
  is therefore: SIGP (s_r = p5(lam5*pr)+1 = 2*sigma(pr); PSUM prz r-half)
  -> ZP7 (zp2 = 1-p7(lam7*pz) = 2*(1-z); PSUM prz z-half, deg-7) -> opB
  (wt = s_r * pnh; PSUM pnh; pnh = 0.5*ghn so wt = sigma(pr)*ghn) -> T1A
  (tanh stage 1 of wt+gxn) -> T2M (tanh stage 2 * zp2) -> b. AM1
  (a = (1-0.5*zp2)*h_prev) and the h=a+b add fill the ack gaps between
  chain ops; everything runs on DVE so no cross-engine semaphores exist
  except PE<->DVE (the tile framework also serializes same-PSUM-bank
  reads across engines, hence no ACT sigmoid).
- The 0.5 scales fold into weights/consumers: MU=0.5 on the n-gate
  recurrent weights, s0=0.5 in AM1, and the tanh composite pair rescaled
  by k = 0.5^(1/7) so t2'(t1'(x))*zp2 = tanh(x)*zp exactly.
- Preload matmuls for the next chunk are split into 128-column quarters
  spread over steps 4..11 so they never stall the PE queue; the gxn
  PSUM->SBUF copy runs on the otherwise-idle ACT engine (GPSIMD cannot
  access PSUM).

End-to-end emulated rel err vs fp64 reference: 3.8e-3 (threshold 2e-2).
"""
import sys

sys.path.insert(0, "/opt/trn_rl_repo")
import numpy as np
from contextlib import ExitStack

import concourse.bass as bass
import concourse.bacc as bacc
import concourse.tile as tile
from concourse import mybir, dve_ops
from concourse.bass_utils import run_bass_kernel_spmd
from concourse.dve_spec import (Spec, Src0, Src1, C0, C1, C2, One, sq, lower,
                                _has_src1)
from concourse.dve_uop import DveOpSpec

F32 = mybir.dt.float32
AF = mybir.ActivationFunctionType
OP = mybir.AluOpType

N_CORES = 8
B_FULL, T, H = 256, 2048, 50
B = B_FULL // N_CORES  # 32 sequences per core
Tc = 16  # steps per PSUM chunk (16*32 = 512 fp32 = one PSUM bank)
C = T // Tc
K = H + 1  # hidden dim + ones row (bias folding)
N = Tc * B
H2 = 2 * H  # merged r|z width
Q = 128  # preload matmul quarter width

# r gate: sigma(x) ~= 0.5*(1 + p5(lam5*x)), p5(y) = y*((y^2+B)^2+C):
# odd deg-5 true-minimax fit of tanh(x/2) on [0, 3.75] (sigma err 4.1e-3;
# preactivation range of this problem is within +-3.7)
SIG5_LAM = 0.2417416084607673
SIG5_B = -0.9782487520328667
SIG5_C = 1.037089534222082
# z gate: zp2 = 1 - p7(lam7*x), p7(y) = y*(y^2+A)*((y^2+B)^2+C): odd
# deg-7 minimax fit of tanh(x/2) on [-4, 4] (sigma err 1.3e-3)
ZP7_LAM = -0.23686290853449668
ZP7_A = -1.659553586057441
ZP7_B = -0.4062605989865425
ZP7_C = 1.0904867013900061
MU = 0.5
# tanh composite (v8 params) rescaled by k = 0.5^(1/7) to absorb zp2 = 2*zp
_K = 0.5 ** (1.0 / 7.0)
TANH_P = [-0.4688424773558226 * _K, 0.028358597211190325 * _K,
          -0.0008122054381782474 * _K,
          -1.595713646067847 * _K * _K, -0.35458487718736414 * _K * _K,
          1.2073796942521828 * _K ** 4]


def _register(name, spec):
    for op in dve_ops.OPS:
        if op.name == name:
            return op
    row = dve_ops._CUSTOM_DVE_ROW_BASE + len(dve_ops.OPS)
    sha = {}
    for ver in ("v3", "v4"):
        tmp = DveOpSpec(name=name, opcode=row, uops=lower(spec, ver=ver),
                        rd1_en=_has_src1(spec))
        sha[ver] = tmp.sha(ver)
    op = dve_ops.DveOp(name, spec, subdim=False, uops_sha=sha)
    dve_ops.OPS.append(op)
    dve_ops._SUB_OPCODE_FOR_NAME[name] = row
    dve_ops.CUSTOM_DVE_SPECS[name] = spec
    return op


def _mk_ops():
    u = sq(Src0)
    # s_r = p5(pr') + 1 = 2*sigma(pr)   [pr' pre-scaled by lam5 via weights]
    sigp = Spec(body=Src0 * (sq(u + C0) + C1) + One,
                reference=lambda in0, in1, s0, s1, imm2:
                in0 * ((in0 * in0 + s0) ** 2 + s1) + 1.0)
    # zp2 = 1 - p7(pz')   [pz' pre-scaled by lam7 via weights]
    zp7 = Spec(body=One - Src0 * (u + C0) * (sq(u + C1) + C2),
               reference=lambda in0, in1, s0, s1, imm2:
               1.0 - in0 * (in0 * in0 + s0) * ((in0 * in0 + s1) ** 2 + imm2))
    # a = (1 - 0.5*zp2) * h_prev
    am1 = Spec(body=(One - Src0 * C0) * Src1,
               reference=lambda in0, in1, s0, s1, imm2: (1.0 - in0 * s0) * in1)
    xx = Src0 + Src1
    ux = sq(xx)
    t1a = Spec(body=xx * ((ux * C0 + C1) * ux + C2),
               reference=lambda in0, in1, s0, s1, imm2:
               (in0 + in1) * (((in0 + in1) ** 2 * s0 + s1) * (in0 + in1) ** 2
                              + imm2))
    w = sq(Src0)
    t2m = Spec(body=Src0 * (w + C0) * (sq(w + C1) + C2) * Src1,
               reference=lambda in0, in1, s0, s1, imm2:
               in0 * (in0 * in0 + s0) * ((in0 * in0 + s1) ** 2 + imm2) * in1)
    return (_register("GRU_SIGP_ANT", sigp), _register("GRU_ZP7_ANT", zp7),
            _register("GRU_AM1S_ANT", am1), _register("GRU_T1A_ANT", t1a),
            _register("GRU_T2M_ANT", t2m))


OP_SIGP, OP_ZP7, OP_AM1, OP_T1A, OP_T2M = _mk_ops()


def _build_nc(repeats=1, n_chunks=C):
    nc = bacc.Bacc("TRN2", target_bir_lowering=False, debug=False,
                   num_devices=N_CORES)
    xt = nc.dram_tensor("xt", (C, K, N), F32, kind="ExternalInput")
    wxr = nc.dram_tensor("wxr", (K, H), F32, kind="ExternalInput")
    wxz = nc.dram_tensor("wxz", (K, H), F32, kind="ExternalInput")
    wxn = nc.dram_tensor("wxn", (K, H), F32, kind="ExternalInput")
    whr = nc.dram_tensor("whr", (K, H), F32, kind="ExternalInput")
    whz = nc.dram_tensor("whz", (K, H), F32, kind="ExternalInput")
    whnp = nc.dram_tensor("whnp", (K, H), F32, kind="ExternalInput")
    h0a = nc.dram_tensor("h0a", (K, B), F32, kind="ExternalInput")
    h0b = nc.dram_tensor("h0b", (K, B), F32, kind="ExternalInput")
    y = nc.dram_tensor("y", (H, B), F32, kind="ExternalOutput")

    sB5, sC5 = SIG5_B, SIG5_C
    zA, zB, zC = ZP7_A, ZP7_B, ZP7_C
    c0t, c1t, c2t = TANH_P[0], TANH_P[1], TANH_P[2]
    a2t, s2t, t2t = TANH_P[3], TANH_P[4], TANH_P[5]

    with ExitStack() as ctx:
        tc_ctx = ctx.enter_context(tile.TileContext(nc))
        consts = ctx.enter_context(tc_ctx.tile_pool(name="consts", bufs=1))
        xpool = ctx.enter_context(tc_ctx.tile_pool(name="xp", bufs=3))
        prp = ctx.enter_context(
            tc_ctx.tile_pool(name="pr", bufs=2, space="PSUM"))
        pzp = ctx.enter_context(
            tc_ctx.tile_pool(name="pz", bufs=2, space="PSUM"))
        pnp = ctx.enter_context(
            tc_ctx.tile_pool(name="pn", bufs=2, space="PSUM"))
        pnhp = ctx.enter_context(
            tc_ctx.tile_pool(name="pnh", bufs=2, space="PSUM"))
        gxp = ctx.enter_context(tc_ctx.tile_pool(name="gx", bufs=2))
        steps = ctx.enter_context(tc_ctx.tile_pool(name="st", bufs=4))

        wxr_sb = consts.tile([K, H], F32, tag="wxr")
        wxz_sb = consts.tile([K, H], F32, tag="wxz")
        wxn_sb = consts.tile([K, H], F32, tag="wxn")
        whr_sb = consts.tile([K, H], F32, tag="whr")
        whz_sb = consts.tile([K, H], F32, tag="whz")
        whnp_sb = consts.tile([K, H], F32, tag="whnp")
        a_sb = consts.tile([K, B], F32, tag="a")
        b_sb = consts.tile([K, B], F32, tag="b")
        h_sb = consts.tile([H, B], F32, tag="h")
        for t_sb, t_dr in ((wxr_sb, wxr), (wxz_sb, wxz), (wxn_sb, wxn),
                           (whr_sb, whr), (whz_sb, whz), (whnp_sb, whnp)):
            nc.sync.dma_start(out=t_sb[:], in_=t_dr[:, :])

        def preload_full(c):
            xt_sb = xpool.tile([K, N], F32, tag="xt")
            nc.sync.dma_start(out=xt_sb[:], in_=xt[c, :, :])
            pr = prp.tile([H, N], F32, tag="pr")
            pz = pzp.tile([H, N], F32, tag="pz")
            pn = pnp.tile([H, N], F32, tag="pn")
            nc.tensor.matmul(pr[:], wxr_sb[:], xt_sb[:], start=True,
                             stop=False, skip_group_check=True)
            nc.tensor.matmul(pz[:], wxz_sb[:], xt_sb[:], start=True,
                             stop=False, skip_group_check=True)
            nc.tensor.matmul(pn[:], wxn_sb[:], xt_sb[:], start=True,
                             stop=True, skip_group_check=True)
            pnx_sb = gxp.tile([H, N], F32, tag="pnx")
            nc.scalar.activation(pnx_sb[:], pn[:], AF.Copy)
            return pr, pz, pnx_sb

        for _rep in range(repeats):
            nc.sync.dma_start(out=a_sb[:], in_=h0a[:, :])
            nc.sync.dma_start(out=b_sb[:], in_=h0b[:, :])
            nc.sync.dma_start(out=h_sb[:], in_=h0b[0:H, :])

            cur = preload_full(0)
            nxt_xt = nxt_pr = nxt_pz = nxt_pn = nxt_pnx = None
            HQ = N // 2
            for c in range(n_chunks):
                pr, pz, pnx_sb = cur
                for ti in range(Tc):
                    more = c + 1 < n_chunks
                    if more and ti == 0:
                        nxt_xt = xpool.tile([K, N], F32, tag="xt")
                        nc.sync.dma_start(out=nxt_xt[:], in_=xt[c + 1, :, :])
                        nxt_pr = prp.tile([H, N], F32, tag="pr")
                        nxt_pz = pzp.tile([H, N], F32, tag="pz")
                        nxt_pn = pnp.tile([H, N], F32, tag="pn")
                    if more and 4 <= ti < 10:
                        half = (ti - 4) % 2
                        w_sb, dst = ((wxr_sb, nxt_pr), (wxz_sb, nxt_pz),
                                     (wxn_sb, nxt_pn))[(ti - 4) // 2]
                        qs = bass.ts(half, HQ)
                        nc.tensor.matmul(dst[:, qs], w_sb[:],
                                         nxt_xt[:, qs], start=True,
                                         stop=(dst is nxt_pn),
                                         skip_group_check=True)
                    if more and ti == 10:
                        nxt_pnx = gxp.tile([H, N], F32, tag="pnx")
                        nc.scalar.activation(nxt_pnx[:], nxt_pn[:], AF.Copy)

                    sl = bass.ts(ti, B)
                    pnh = pnhp.tile([H, B], F32, tag="pnh")
                    # a-dependent matmuls first: a lands mid-previous-step,
                    # so these drain from the in-order PE queue early
                    nc.tensor.matmul(pr[:, sl], whr_sb[:], a_sb[:],
                                     start=False, stop=False,
                                     skip_group_check=True)
                    nc.tensor.matmul(pz[:, sl], whz_sb[:], a_sb[:],
                                     start=False, stop=False,
                                     skip_group_check=True)
                    nc.tensor.matmul(pnh[:], whnp_sb[:], a_sb[:], start=True,
                                     stop=False, skip_group_check=True)
                    # b-dependent matmuls (critical): pr stop first gates
                    # SIGP. pz_b LAST so ZP7's attached pz-wait carries the
                    # highest PE sem count, subsuming wt's pnh dependency --
                    # wt then has a single (DVE-ack) wait and pre-decodes
                    # instead of blocking the sequencer on a standalone
                    # EventSemaphore.
                    nc.tensor.matmul(pr[:, sl], whr_sb[:], b_sb[:],
                                     start=False, stop=True,
                                     skip_group_check=True)
                    nc.tensor.matmul(pnh[:], whnp_sb[:], b_sb[:], start=False,
                                     stop=True, skip_group_check=True)
                    nc.tensor.matmul(pz[:, sl], whz_sb[:], b_sb[:],
                                     start=False, stop=True,
                                     skip_group_check=True)
                    # DVE chain (each op reads at most one PSUM input);
                    # AM1/h-add fill the write-ack gaps between chain ops
                    sr = steps.tile([H, B], F32, tag="sr")
                    nc.vector._custom_dve(OP_SIGP, out=sr[:],
                                          in0=pr[:, sl], s0=sB5, s1=sC5)
                    zp2 = steps.tile([H, B], F32, tag="zp2")
                    nc.vector._custom_dve(OP_ZP7, out=zp2[:],
                                          in0=pz[:, sl],
                                          s0=zA, s1=zB, imm2=zC)
                    wt = steps.tile([H, B], F32, tag="wt")
                    nc.vector.tensor_tensor(wt[:], sr[:], pnh[:], op=OP.mult)
                    nc.vector._custom_dve(OP_AM1, out=a_sb[0:H, :],
                                          in0=zp2[:], in1=h_sb[:], s0=0.5)
                    yn = steps.tile([H, B], F32, tag="yn")
                    nc.vector._custom_dve(OP_T1A, out=yn[:], in0=wt[:],
                                          in1=pnx_sb[:, sl],
                                          s0=c2t, s1=c1t, imm2=c0t)
                    nc.vector._custom_dve(OP_T2M, out=b_sb[0:H, :], in0=yn[:],
                                          in1=zp2[:], s0=a2t, s1=s2t,
                                          imm2=t2t)
                    nc.vector.tensor_tensor(h_sb[:], a_sb[0:H, :],
                                            b_sb[0:H, :], op=OP.add)
                if c + 1 < n_chunks:
                    cur = (nxt_pr, nxt_pz, nxt_pnx)
        nc.sync.dma_start(out=y[:, :], in_=h_sb[:])
    nc.compile()
    return nc


def _prep_in_maps(inputs, W_ih, W_hh, b_ih, b_hh):
    inputs = np.ascontiguousarray(inputs, dtype=np.float32)
    W_ih = np.asarray(W_ih, dtype=np.float32)
    W_hh = np.asarray(W_hh, dtype=np.float32)
    b_ih = np.asarray(b_ih, dtype=np.float32)
    b_hh = np.asarray(b_hh, dtype=np.float32)
    lam5 = np.float32(SIG5_LAM)
    lam7 = np.float32(ZP7_LAM)
    mu = np.float32(MU)

    def wx(gate, lam):
        w = np.empty((K, H), np.float32)
        w[0:H] = lam * W_ih[gate * H:(gate + 1) * H].T
        w[H] = lam * (b_ih[gate * H:(gate + 1) * H]
                      + b_hh[gate * H:(gate + 1) * H])
        return w

    def wh(gate, lam):
        w = np.zeros((K, H), np.float32)
        w[0:H] = lam * W_hh[gate * H:(gate + 1) * H].T
        return w

    wxr, wxz = wx(0, lam5), wx(1, lam7)
    whr, whz = wh(0, lam5), wh(1, lam7)

    wxn = np.empty((K, H), np.float32)
    wxn[0:H] = W_ih[H2:].T
    wxn[H] = b_ih[H2:]

    whnp = np.zeros((K, H), np.float32)
    whnp[0:H] = mu * W_hh[H2:].T
    whnp[H] = mu * b_hh[H2:]

    h0a = np.zeros((K, B), np.float32)
    h0a[H] = 1.0
    h0b = np.zeros((K, B), np.float32)

    in_maps = []
    for core in range(N_CORES):
        xc = inputs[core * B:(core + 1) * B]  # (B, T, H)
        xa = np.concatenate([xc, np.ones((B, T, 1), np.float32)], axis=2)
        xtc = np.ascontiguousarray(
            xa.reshape(B, C, Tc, K).transpose(1, 3, 2, 0).reshape(C, K, N))
        in_maps.append({"xt": xtc, "wxr": wxr, "wxz": wxz, "wxn": wxn,
                        "whr": whr, "whz": whz, "whnp": whnp,
                        "h0a": h0a, "h0b": h0b})
    return in_maps


_NC_CACHE = []


def kernel(inputs, W_ih, W_hh, b_ih, b_hh, z=0, **_ignored):
    if np.asarray(inputs).ndim == 2:
        inputs = np.asarray(inputs)[None]
    if not _NC_CACHE:
        _NC_CACHE.append(_build_nc())
    nc = _NC_CACHE[0]
    in_maps = _prep_in_maps(inputs, W_ih, W_hh, b_ih, b_hh)
    res = run_bass_kernel_spmd(nc, in_maps, core_ids=list(range(N_CORES)))
    out = np.empty((B_FULL, H), np.float32)
    for core in range(N_CORES):
        out[core * B:(core + 1) * B] = res.results[core]["y"].T
    return out


if __name__ == "__main__":
    rng = np.random.default_rng(0)
    s = 1.0 / np.sqrt(H)
    demo = {
        "inputs": rng.standard_normal((B_FULL, T, H), dtype=np.float32),
        "W_ih": rng.uniform(-s, s, (3 * H, H)).astype(np.float32),
        "W_hh": rng.uniform(-s, s, (3 * H, H)).astype(np.float32),
        "b_ih": rng.uniform(-s, s, (3 * H,)).astype(np.float32),
        "b_hh": rng.uniform(-s, s, (3 * H,)).astype(np.float32),
        "z": 0,
    }
    out = kernel(**demo)
    print("kernel output", out.shape, out.dtype, out[0, :4])


# revision 15
# speedup vs baseline: 1.0141x; 1.0141x over previous
"""Trainium2 Bass GRU kernel v13: all-DVE recurrent chain, legal PSUM reads.

Per core (32 sequences), transposed layout (hidden on partitions, batch on
free dim). State kept as a = z*h_prev and b = (1-z)*n so PSUM accumulation
performs h = a + b inside the recurrent matmuls. Design:

- r, z, n-recurrent preactivations live in SEPARATE PSUM banks: the tile
  framework sem-orders reads of the same PSUM tile (even on one engine,
  with the writer ack), so sharing a bank between SIGP and ZP7 costs
  160ns/step. 6 recurrent matmuls/step; PE is far from busy.
- HW constraint: a DVE op may read at most ONE input from PSUM. The chain
  is therefore: SIGP (s_r = p5(lam5*pr)+1 = 2*sigma(pr); PSUM prz r-half)
  -> ZP7 (zp2 = 1-p7(lam7*pz) = 2*(1-z); PSUM prz z-half, deg-7) -> opB
  (wt = s_r * pnh; PSUM pnh; pnh = 0.5*ghn so wt = sigma(pr)*ghn) -> T1A
  (tanh stage 1 of wt+gxn) -> T2M (tanh stage 2 * zp2) -> b. AM1
  (a = (1-0.5*zp2)*h_prev) and the h=a+b add fill the ack gaps between
  chain ops; everything runs on DVE so no cross-engine semaphores exist
  except PE<->DVE (the tile framework also serializes same-PSUM-bank
  reads across engines, hence no ACT sigmoid).
- The 0.5 scales fold into weights/consumers: MU=0.5 on the n-gate
  recurrent weights, s0=0.5 in AM1, and the tanh composite pair rescaled
  by k = 0.5^(1/7) so t2'(t1'(x))*zp2 = tanh(x)*zp exactly.
- Preload matmuls for the next chunk are split into 128-column quarters
  spread over steps 4..11 so they never stall the PE queue; the gxn
  PSUM->SBUF copy runs on the otherwise-idle ACT engine (GPSIMD cannot
  access PSUM).

End-to-end emulated rel err vs fp64 reference: 3.8e-3 (threshold 2e-2).
"""
import sys

sys.path.insert(0, "/opt/trn_rl_repo")
import numpy as np
from contextlib import ExitStack

import concourse.bass as bass
import concourse.bacc as bacc
import concourse.tile as tile
from concourse import mybir, dve_ops
from concourse.bass_utils import run_bass_kernel_spmd
from concourse.dve_spec import (Spec, Src0, Src1, C0, C1, C2, One, sq, lower,
                                _has_src1)
from concourse.dve_uop import DveOpSpec

F32 = mybir.dt.float32
AF = mybir.ActivationFunctionType
OP = mybir.AluOpType

N_CORES = 8
B_FULL, T, H = 256, 2048, 50
B = B_FULL // N_CORES  # 32 sequences per core
Tc = 16  # steps per PSUM chunk (16*32 = 512 fp32 = one PSUM bank)
C = T // Tc
K = H + 1  # hidden dim + ones row (bias folding)
N = Tc * B
H2 = 2 * H  # merged r|z width
Q = 128  # preload matmul quarter width

# r gate: sigma(x) ~= 0.5*(1 + p5(lam5*x)), p5(y) = y*((y^2+B)^2+C):
# odd deg-5 true-minimax fit of tanh(x/2) on [0, 3.75] (sigma err 4.1e-3;
# preactivation range of this problem is within +-3.7)
SIG5_LAM = 0.2417416084607673
SIG5_B = -0.9782487520328667
SIG5_C = 1.037089534222082
# z gate: zp2 = 1 - p7(lam7*x), p7(y) = y*(y^2+A)*((y^2+B)^2+C): odd
# deg-7 minimax fit of tanh(x/2) on [-4, 4] (sigma err 1.3e-3)
ZP7_LAM = -0.23686290853449668
ZP7_A = -1.659553586057441
ZP7_B = -0.4062605989865425
ZP7_C = 1.0904867013900061
MU = 0.5
# tanh composite (v8 params) rescaled by k = 0.5^(1/7) to absorb zp2 = 2*zp
_K = 0.5 ** (1.0 / 7.0)
TANH_P = [-0.4688424773558226 * _K, 0.028358597211190325 * _K,
          -0.0008122054381782474 * _K,
          -1.595713646067847 * _K * _K, -0.35458487718736414 * _K * _K,
          1.2073796942521828 * _K ** 4]


def _register(name, spec):
    for op in dve_ops.OPS:
        if op.name == name:
            return op
    row = dve_ops._CUSTOM_DVE_ROW_BASE + len(dve_ops.OPS)
    sha = {}
    for ver in ("v3", "v4"):
        tmp = DveOpSpec(name=name, opcode=row, uops=lower(spec, ver=ver),
                        rd1_en=_has_src1(spec))
        sha[ver] = tmp.sha(ver)
    op = dve_ops.DveOp(name, spec, subdim=False, uops_sha=sha)
    dve_ops.OPS.append(op)
    dve_ops._SUB_OPCODE_FOR_NAME[name] = row
    dve_ops.CUSTOM_DVE_SPECS[name] = spec
    return op


def _mk_ops():
    u = sq(Src0)
    # s_r = p5(pr') + 1 = 2*sigma(pr)   [pr' pre-scaled by lam5 via weights]
    sigp = Spec(body=Src0 * (sq(u + C0) + C1) + One,
                reference=lambda in0, in1, s0, s1, imm2:
                in0 * ((in0 * in0 + s0) ** 2 + s1) + 1.0)
    # zp2 = 1 - p7(pz')   [pz' pre-scaled by lam7 via weights]
    zp7 = Spec(body=One - Src0 * (u + C0) * (sq(u + C1) + C2),
               reference=lambda in0, in1, s0, s1, imm2:
               1.0 - in0 * (in0 * in0 + s0) * ((in0 * in0 + s1) ** 2 + imm2))
    # a = (1 - 0.5*zp2) * h_prev
    am1 = Spec(body=(One - Src0 * C0) * Src1,
               reference=lambda in0, in1, s0, s1, imm2: (1.0 - in0 * s0) * in1)
    xx = Src0 + Src1
    ux = sq(xx)
    t1a = Spec(body=xx * ((ux * C0 + C1) * ux + C2),
               reference=lambda in0, in1, s0, s1, imm2:
               (in0 + in1) * (((in0 + in1) ** 2 * s0 + s1) * (in0 + in1) ** 2
                              + imm2))
    w = sq(Src0)
    t2m = Spec(body=Src0 * (w + C0) * (sq(w + C1) + C2) * Src1,
               reference=lambda in0, in1, s0, s1, imm2:
               in0 * (in0 * in0 + s0) * ((in0 * in0 + s1) ** 2 + imm2) * in1)
    return (_register("GRU_SIGP_ANT", sigp), _register("GRU_ZP7_ANT", zp7),
            _register("GRU_AM1S_ANT", am1), _register("GRU_T1A_ANT", t1a),
            _register("GRU_T2M_ANT", t2m))


OP_SIGP, OP_ZP7, OP_AM1, OP_T1A, OP_T2M = _mk_ops()


def _build_nc(repeats=1, n_chunks=C):
    nc = bacc.Bacc("TRN2", target_bir_lowering=False, debug=False,
                   num_devices=N_CORES)
    xt = nc.dram_tensor("xt", (C, K, N), F32, kind="ExternalInput")
    wxr = nc.dram_tensor("wxr", (K, H), F32, kind="ExternalInput")
    wxz = nc.dram_tensor("wxz", (K, H), F32, kind="ExternalInput")
    wxn = nc.dram_tensor("wxn", (K, H), F32, kind="ExternalInput")
    whr = nc.dram_tensor("whr", (K, H), F32, kind="ExternalInput")
    whz = nc.dram_tensor("whz", (K, H), F32, kind="ExternalInput")
    whnp = nc.dram_tensor("whnp", (K, H), F32, kind="ExternalInput")
    h0a = nc.dram_tensor("h0a", (K, B), F32, kind="ExternalInput")
    h0b = nc.dram_tensor("h0b", (K, B), F32, kind="ExternalInput")
    y = nc.dram_tensor("y", (H, B), F32, kind="ExternalOutput")

    sB5, sC5 = SIG5_B, SIG5_C
    zA, zB, zC = ZP7_A, ZP7_B, ZP7_C
    c0t, c1t, c2t = TANH_P[0], TANH_P[1], TANH_P[2]
    a2t, s2t, t2t = TANH_P[3], TANH_P[4], TANH_P[5]

    with ExitStack() as ctx:
        tc_ctx = ctx.enter_context(tile.TileContext(nc))
        consts = ctx.enter_context(tc_ctx.tile_pool(name="consts", bufs=1))
        xpool = ctx.enter_context(tc_ctx.tile_pool(name="xp", bufs=3))
        prp = ctx.enter_context(
            tc_ctx.tile_pool(name="pr", bufs=2, space="PSUM"))
        pzp = ctx.enter_context(
            tc_ctx.tile_pool(name="pz", bufs=2, space="PSUM"))
        pnp = ctx.enter_context(
            tc_ctx.tile_pool(name="pn", bufs=2, space="PSUM"))
        pnhp = ctx.enter_context(
            tc_ctx.tile_pool(name="pnh", bufs=2, space="PSUM"))
        gxp = ctx.enter_context(tc_ctx.tile_pool(name="gx", bufs=2))
        steps = ctx.enter_context(tc_ctx.tile_pool(name="st", bufs=4))

        wxr_sb = consts.tile([K, H], F32, tag="wxr")
        wxz_sb = consts.tile([K, H], F32, tag="wxz")
        wxn_sb = consts.tile([K, H], F32, tag="wxn")
        whr_sb = consts.tile([K, H], F32, tag="whr")
        whz_sb = consts.tile([K, H], F32, tag="whz")
        whnp_sb = consts.tile([K, H], F32, tag="whnp")
        a_sb = consts.tile([K, B], F32, tag="a")
        b_sb = consts.tile([K, B], F32, tag="b")
        h_sb = consts.tile([H, B], F32, tag="h")
        for t_sb, t_dr in ((wxr_sb, wxr), (wxz_sb, wxz), (wxn_sb, wxn),
                           (whr_sb, whr), (whz_sb, whz), (whnp_sb, whnp)):
            nc.sync.dma_start(out=t_sb[:], in_=t_dr[:, :])

        def preload_full(c):
            xt_sb = xpool.tile([K, N], F32, tag="xt")
            nc.sync.dma_start(out=xt_sb[:], in_=xt[c, :, :])
            pr = prp.tile([H, N], F32, tag="pr")
            pz = pzp.tile([H, N], F32, tag="pz")
            pn = pnp.tile([H, N], F32, tag="pn")
            nc.tensor.matmul(pr[:], wxr_sb[:], xt_sb[:], start=True,
                             stop=False, skip_group_check=True)
            nc.tensor.matmul(pz[:], wxz_sb[:], xt_sb[:], start=True,
                             stop=False, skip_group_check=True)
            nc.tensor.matmul(pn[:], wxn_sb[:], xt_sb[:], start=True,
                             stop=True, skip_group_check=True)
            pnx_sb = gxp.tile([H, N], F32, tag="pnx")
            nc.scalar.activation(pnx_sb[:], pn[:], AF.Copy)
            return pr, pz, pnx_sb

        for _rep in range(repeats):
            nc.sync.dma_start(out=a_sb[:], in_=h0a[:, :])
            nc.sync.dma_start(out=b_sb[:], in_=h0b[:, :])
            nc.sync.dma_start(out=h_sb[:], in_=h0b[0:H, :])

            cur = preload_full(0)
            nxt_xt = nxt_pr = nxt_pz = nxt_pn = nxt_pnx = None
            HQ = N // 2
            for c in range(n_chunks):
                pr, pz, pnx_sb = cur
                for ti in range(Tc):
                    more = c + 1 < n_chunks
                    if more and ti == 0:
                        nxt_xt = xpool.tile([K, N], F32, tag="xt")
                        nc.sync.dma_start(out=nxt_xt[:], in_=xt[c + 1, :, :])
                        nxt_pr = prp.tile([H, N], F32, tag="pr")
                        nxt_pz = pzp.tile([H, N], F32, tag="pz")
                        nxt_pn = pnp.tile([H, N], F32, tag="pn")
                    if more and ti in (4, 6, 8):
                        w_sb, dst = ((wxr_sb, nxt_pr), (wxz_sb, nxt_pz),
                                     (wxn_sb, nxt_pn))[(ti - 4) // 2]
                        nc.tensor.matmul(dst[:], w_sb[:], nxt_xt[:],
                                         start=True,
                                         stop=(dst is nxt_pn),
                                         skip_group_check=True)
                    if more and ti == 10:
                        nxt_pnx = gxp.tile([H, N], F32, tag="pnx")
                        nc.scalar.activation(nxt_pnx[:], nxt_pn[:], AF.Copy)

                    sl = bass.ts(ti, B)
                    pnh = pnhp.tile([H, B], F32, tag="pnh")
                    # a-dependent matmuls first: a lands mid-previous-step,
                    # so these drain from the in-order PE queue early
                    nc.tensor.matmul(pr[:, sl], whr_sb[:], a_sb[:],
                                     start=False, stop=False,
                                     skip_group_check=True)
                    nc.tensor.matmul(pz[:, sl], whz_sb[:], a_sb[:],
                                     start=False, stop=False,
                                     skip_group_check=True)
                    nc.tensor.matmul(pnh[:], whnp_sb[:], a_sb[:], start=True,
                                     stop=False, skip_group_check=True)
                    # b-dependent matmuls (critical): pr stop first gates
                    # SIGP. pz_b LAST so ZP7's attached pz-wait carries the
                    # highest PE sem count, subsuming wt's pnh dependency --
                    # wt then has a single (DVE-ack) wait and pre-decodes
                    # instead of blocking the sequencer on a standalone
                    # EventSemaphore.
                    nc.tensor.matmul(pr[:, sl], whr_sb[:], b_sb[:],
                                     start=False, stop=True,
                                     skip_group_check=True)
                    nc.tensor.matmul(pnh[:], whnp_sb[:], b_sb[:], start=False,
                                     stop=True, skip_group_check=True)
                    nc.tensor.matmul(pz[:, sl], whz_sb[:], b_sb[:],
                                     start=False, stop=True,
                                     skip_group_check=True)
                    # DVE chain (each op reads at most one PSUM input);
                    # AM1/h-add fill the write-ack gaps between chain ops
                    sr = steps.tile([H, B], F32, tag="sr")
                    nc.vector._custom_dve(OP_SIGP, out=sr[:],
                                          in0=pr[:, sl], s0=sB5, s1=sC5)
                    zp2 = steps.tile([H, B], F32, tag="zp2")
                    nc.vector._custom_dve(OP_ZP7, out=zp2[:],
                                          in0=pz[:, sl],
                                          s0=zA, s1=zB, imm2=zC)
                    wt = steps.tile([H, B], F32, tag="wt")
                    nc.vector.tensor_tensor(wt[:], sr[:], pnh[:], op=OP.mult)
                    nc.vector._custom_dve(OP_AM1, out=a_sb[0:H, :],
                                          in0=zp2[:], in1=h_sb[:], s0=0.5)
                    yn = steps.tile([H, B], F32, tag="yn")
                    nc.vector._custom_dve(OP_T1A, out=yn[:], in0=wt[:],
                                          in1=pnx_sb[:, sl],
                                          s0=c2t, s1=c1t, imm2=c0t)
                    nc.vector._custom_dve(OP_T2M, out=b_sb[0:H, :], in0=yn[:],
                                          in1=zp2[:], s0=a2t, s1=s2t,
                                          imm2=t2t)
                    nc.vector.tensor_tensor(h_sb[:], a_sb[0:H, :],
                                            b_sb[0:H, :], op=OP.add)
                if c + 1 < n_chunks:
                    cur = (nxt_pr, nxt_pz, nxt_pnx)
        nc.sync.dma_start(out=y[:, :], in_=h_sb[:])
    nc.compile()
    return nc


def _prep_in_maps(inputs, W_ih, W_hh, b_ih, b_hh):
    inputs = np.ascontiguousarray(inputs, dtype=np.float32)
    W_ih = np.asarray(W_ih, dtype=np.float32)
    W_hh = np.asarray(W_hh, dtype=np.float32)
    b_ih = np.asarray(b_ih, dtype=np.float32)
    b_hh = np.asarray(b_hh, dtype=np.float32)
    lam5 = np.float32(SIG5_LAM)
    lam7 = np.float32(ZP7_LAM)
    mu = np.float32(MU)

    def wx(gate, lam):
        w = np.empty((K, H), np.float32)
        w[0:H] = lam * W_ih[gate * H:(gate + 1) * H].T
        w[H] = lam * (b_ih[gate * H:(gate + 1) * H]
                      + b_hh[gate * H:(gate + 1) * H])
        return w

    def wh(gate, lam):
        w = np.zeros((K, H), np.float32)
        w[0:H] = lam * W_hh[gate * H:(gate + 1) * H].T
        return w

    wxr, wxz = wx(0, lam5), wx(1, lam7)
    whr, whz = wh(0, lam5), wh(1, lam7)

    wxn = np.empty((K, H), np.float32)
    wxn[0:H] = W_ih[H2:].T
    wxn[H] = b_ih[H2:]

    whnp = np.zeros((K, H), np.float32)
    whnp[0:H] = mu * W_hh[H2:].T
    whnp[H] = mu * b_hh[H2:]

    h0a = np.zeros((K, B), np.float32)
    h0a[H] = 1.0
    h0b = np.zeros((K, B), np.float32)

    in_maps = []
    for core in range(N_CORES):
        xc = inputs[core * B:(core + 1) * B]  # (B, T, H)
        xa = np.concatenate([xc, np.ones((B, T, 1), np.float32)], axis=2)
        xtc = np.ascontiguousarray(
            xa.reshape(B, C, Tc, K).transpose(1, 3, 2, 0).reshape(C, K, N))
        in_maps.append({"xt": xtc, "wxr": wxr, "wxz": wxz, "wxn": wxn,
                        "whr": whr, "whz": whz, "whnp": whnp,
                        "h0a": h0a, "h0b": h0b})
    return in_maps


_NC_CACHE = []


def kernel(inputs, W_ih, W_hh, b_ih, b_hh, z=0, **_ignored):
    if np.asarray(inputs).ndim == 2:
        inputs = np.asarray(inputs)[None]
    if not _NC_CACHE:
        _NC_CACHE.append(_build_nc())
    nc = _NC_CACHE[0]
    in_maps = _prep_in_maps(inputs, W_ih, W_hh, b_ih, b_hh)
    res = run_bass_kernel_spmd(nc, in_maps, core_ids=list(range(N_CORES)))
    out = np.empty((B_FULL, H), np.float32)
    for core in range(N_CORES):
        out[core * B:(core + 1) * B] = res.results[core]["y"].T
    return out


if __name__ == "__main__":
    rng = np.random.default_rng(0)
    s = 1.0 / np.sqrt(H)
    demo = {
        "inputs": rng.standard_normal((B_FULL, T, H), dtype=np.float32),
        "W_ih": rng.uniform(-s, s, (3 * H, H)).astype(np.float32),
        "W_hh": rng.uniform(-s, s, (3 * H, H)).astype(np.float32),
        "b_ih": rng.uniform(-s, s, (3 * H,)).astype(np.float32),
        "b_hh": rng.uniform(-s, s, (3 * H,)).astype(np.float32),
        "z": 0,
    }
    out = kernel(**demo)
    print("kernel output", out.shape, out.dtype, out[0, :4])
